# revision 15
# baseline (speedup 1.0000x reference)
"""Trainium2 Bass kernel for nn_DecoderBlock (linear-attention decoder block).

Sharding: token-parallel across 8 cores (each core owns (B*T)/8 = 256 rows of
the flattened [B*T, C] token stream; weights replicated per core). The linear
attention is computed exactly via an intra-chunk causal block plus cross-core
KV prefix states; one small fp16 AllGather per batch-group of 4 cores carries
per-core KV states and Kf sums for both attentions. Activations are kept
transposed ([C partitions, tokens free]) so every GEMM lhsT is a plain DRAM
weight slice; V projections run activation-stationary so V emerges in natural
[token, head-dim] layout (no transposes before the K^T V state matmuls).
Elementwise feature chains (elu / rope) operate on [128, 2048] wide tiles.
All layout transposes go through the DMA XBAR (fp16), none through the PE.

Self-contained: only needs numpy + the concourse (Bass) runtime environment.
"""

import math
import numpy as np
from dataclasses import dataclass

P = 128
HD = 64  # head dim (fixed: C // n_head)
LN_EPS = 1e-5


@dataclass(frozen=True)
class Cfg:
    B: int = 2
    T: int = 1024
    C: int = 1024
    H: int = 16
    NCORE: int = 8
    mm: str = "fp16"  # GEMM dtype: fp16 | bf16
    gelu: str = "table"
    debug_dump: bool = False

    @property
    def R(self):
        return self.B * self.T // self.NCORE

    @property
    def KC(self):
        return self.C // P

    @property
    def NT(self):
        return self.R // P

    @property
    def NPAIR(self):
        return self.H // 2

    @property
    def GS(self):  # collective group size (cores per batch)
        return self.NCORE // self.B

    @property
    def AGW(self):
        return 2 * (HD * self.NPAIR + self.NPAIR)


# ---------------------------------------------------------------------------
# Host-side helpers
# ---------------------------------------------------------------------------

def _rope_tables(T):
    inv = 1.0 / (10000.0 ** (np.arange(0, HD, 2, dtype=np.float64) / HD))
    freqs = np.outer(np.arange(T), inv)
    emb = np.concatenate([freqs, freqs], axis=-1)
    return np.cos(emb).astype(np.float32), np.sin(emb).astype(np.float32)


# Head-dim permutation making rotate_half local to 32-partition quadrants
# (so the device can do it with one DVE stream_shuffle). new_row i <- old
# row _PERM[i]; rotation partner of new row i is i±16 within its quadrant.
_PERM = np.concatenate([np.arange(0, 16), np.arange(32, 48),
                        np.arange(16, 32), np.arange(48, 64)])


def _permute_heads(mat, base, nheads, axis):
    idx = np.concatenate([base + h * HD + _PERM for h in range(nheads)])
    full = np.arange(mat.shape[axis])
    full[base:base + nheads * HD] = idx
    return np.take(mat, full, axis=axis)


def _pack_cols(vecs):
    flat = np.concatenate([np.asarray(v, np.float32).ravel() for v in vecs])
    assert flat.size % P == 0
    return np.ascontiguousarray(flat.reshape(-1, P).T)


def _np_wdt(mm):
    if mm == "fp16":
        return np.float16
    import ml_dtypes
    return ml_dtypes.bfloat16


def _host_inputs(cfg: Cfg, inputs):
    B, T, C, NC = cfg.B, cfg.T, cfg.C, cfg.NCORE
    R, NPAIR, GS = cfg.R, cfg.NPAIR, cfg.GS
    wdt = _np_wdt(cfg.mm)
    xf = np.asarray(inputs["x"], np.float32).reshape(B * T, C)
    mf = np.asarray(inputs["memory"], np.float32).reshape(B * T, C)
    cos, sin = _rope_tables(T)

    H = cfg.H
    # permute head dims of roped projections (Q, K, cross Q, cross K)
    qkv_w = _permute_heads(np.asarray(inputs["sa_qkv_w"], np.float32),
                           0, 2 * H, axis=1)
    qkv_b = _permute_heads(np.asarray(inputs["sa_qkv_b"], np.float32),
                           0, 2 * H, axis=0)
    caq_w = _permute_heads(np.asarray(inputs["ca_q_w"], np.float32),
                           0, H, axis=1)
    caq_b = _permute_heads(np.asarray(inputs["ca_q_b"], np.float32),
                           0, H, axis=0)
    cakv_w = _permute_heads(np.asarray(inputs["ca_kv_w"], np.float32),
                            0, H, axis=1)
    cakv_b = _permute_heads(np.asarray(inputs["ca_kv_b"], np.float32),
                            0, H, axis=0)

    params = _pack_cols([
        inputs["ln1_g"], inputs["ln1_b"], inputs["ln2_g"], inputs["ln2_b"],
        inputs["ln3_g"], inputs["ln3_b"],
        qkv_b, inputs["sa_proj_b"], caq_b, cakv_b, inputs["ca_proj_b"],
        inputs["fc_b"], inputs["fcp_b"]])

    tri = np.triu(np.ones((R, R), np.float32))
    maskw = np.ascontiguousarray(np.hstack([tri, tri]).astype(wdt))

    vrows = np.ascontiguousarray(np.stack([
        np.asarray(inputs["sa_qkv_b"], np.float32)[2 * C:3 * C],
        np.asarray(inputs["ca_kv_b"], np.float32)[C:2 * C],
    ]).astype(wdt))

    weights = {k: np.ascontiguousarray(np.asarray(inputs[k]).astype(wdt))
               for k in ("sa_proj_w", "ca_proj_w", "fc_w", "fcp_w")}
    weights["sa_qkv_w"] = np.ascontiguousarray(qkv_w.astype(wdt))
    weights["ca_q_w"] = np.ascontiguousarray(caq_w.astype(wdt))
    weights["ca_kv_w"] = np.ascontiguousarray(cakv_w.astype(wdt))

    in_maps = []
    for c in range(NC):
        r0 = c * R
        pos = np.arange(r0, r0 + R) % T
        cosP = cos[pos].T[_PERM]          # [64, R], permuted dim order
        sinP = sin[pos].T[_PERM].copy()
        sinP[(np.arange(HD) % 32) < 16] *= -1.0   # sign of rotate_half
        cos2 = np.vstack([cosP, cosP])
        sin2 = np.vstack([sinP, sinP])
        cosw = np.ascontiguousarray(np.tile(cos2, (1, NPAIR)).astype(wdt))
        sinw = np.ascontiguousarray(np.tile(sin2, (1, NPAIR)).astype(wdt))
        b = c // GS
        wpre = np.array([1.0 if (r // GS == b and r < c) else 0.0
                         for r in range(NC)], np.float32)
        wtot = np.array([1.0 if r // GS == b else 0.0
                         for r in range(NC)], np.float32)
        wsel = np.ascontiguousarray(
            np.tile(np.concatenate([wpre, wtot])[None, :], (P, 1)))
        m = dict(weights)
        m.update({
            "xh": np.ascontiguousarray(xf[r0:r0 + R].astype(wdt)),
            "mh": np.ascontiguousarray(mf[r0:r0 + R].astype(wdt)),
            "cosw": cosw, "sinw": sinw, "maskw": maskw,
            "vrows": vrows, "wsel": wsel, "params": params,
        })
        in_maps.append(m)
    return in_maps


# ---------------------------------------------------------------------------
# Bass program
# ---------------------------------------------------------------------------

def build_program(cfg: Cfg):
    import concourse.bass as bass
    import concourse.mybir as mybir
    import concourse.tile as tile
    from concourse import bacc
    from concourse.masks import make_identity
    from contextlib import ExitStack

    dt = mybir.dt
    f32 = dt.float32
    F16 = {"fp16": dt.float16, "bf16": dt.bfloat16}[cfg.mm]
    AF = mybir.ActivationFunctionType
    OP = mybir.AluOpType

    B, T, C, H, NC = cfg.B, cfg.T, cfg.C, cfg.H, cfg.NCORE
    R, KC, NT, NPAIR, GS, AGW = cfg.R, cfg.KC, cfg.NT, cfg.NPAIR, cfg.GS, cfg.AGW
    WND = NPAIR * R      # wide tile free width for per-pair packed features
    XW = KC * R          # wide tile free width for the residual stream
    GW = 4               # GEMM m-group width (PSUM banks)

    nc = bacc.Bacc("TRN2", target_bir_lowering=False, debug=False,
                   num_devices=cfg.NCORE)

    xh_d = nc.dram_tensor("xh", [R, C], F16, kind="ExternalInput")
    mh_d = nc.dram_tensor("mh", [R, C], F16, kind="ExternalInput")
    cosw_d = nc.dram_tensor("cosw", [P, WND], F16, kind="ExternalInput")
    sinw_d = nc.dram_tensor("sinw", [P, WND], F16, kind="ExternalInput")
    maskw_d = nc.dram_tensor("maskw", [R, 2 * R], F16, kind="ExternalInput")
    vrows_d = nc.dram_tensor("vrows", [2, C], F16, kind="ExternalInput")
    wsel_d = nc.dram_tensor("wsel", [P, 2 * NC], f32, kind="ExternalInput")
    NPCOL = 19 * KC
    params_d = nc.dram_tensor("params", [P, NPCOL], f32, kind="ExternalInput")
    Wqkv = nc.dram_tensor("sa_qkv_w", [C, 3 * C], F16, kind="ExternalInput")
    Wsap = nc.dram_tensor("sa_proj_w", [C, C], F16, kind="ExternalInput")
    Wcaq = nc.dram_tensor("ca_q_w", [C, C], F16, kind="ExternalInput")
    Wcakv = nc.dram_tensor("ca_kv_w", [C, 2 * C], F16, kind="ExternalInput")
    Wcap = nc.dram_tensor("ca_proj_w", [C, C], F16, kind="ExternalInput")
    Wfc = nc.dram_tensor("fc_w", [C, 4 * C], F16, kind="ExternalInput")
    Wfcp = nc.dram_tensor("fcp_w", [4 * C, C], F16, kind="ExternalInput")
    out_d = nc.dram_tensor("out", [R, C], F16, kind="ExternalOutput")

    off = {}
    cur = 0
    for pname, w in (("ln1_g", KC), ("ln1_b", KC), ("ln2_g", KC), ("ln2_b", KC),
                     ("ln3_g", KC), ("ln3_b", KC), ("qkv_b", 3 * KC),
                     ("sap_b", KC), ("caq_b", KC), ("cakv_b", 2 * KC),
                     ("cap_b", KC), ("fc_b", 4 * KC), ("fcp_b", KC)):
        off[pname] = cur
        cur += w
    assert cur == NPCOL

    with tile.TileContext(nc) as tc, ExitStack() as ctx:
        const = ctx.enter_context(tc.tile_pool(name="const", bufs=1))
        act = ctx.enter_context(tc.tile_pool(name="act", bufs=1))
        hpool = ctx.enter_context(tc.tile_pool(name="hpool", bufs=2))
        wpool = ctx.enter_context(tc.tile_pool(name="wpool", bufs=10))
        tmp = ctx.enter_context(tc.tile_pool(name="tmp", bufs=2))
        gps = ctx.enter_context(tc.tile_pool(name="gps", bufs=7, space="PSUM"))
        sps = gps
        tps = ctx.enter_context(tc.tile_pool(name="tps", bufs=1, space="PSUM"))
        dram = ctx.enter_context(tc.tile_pool(name="dram", bufs=1, space="DRAM"))

        # ---------------- constants ----------------
        params = const.tile([P, NPCOL], f32, name="params")
        nc.sync.dma_start(params[:], params_d[:, :])
        wsel = const.tile([P, 2 * NC], f32, name="wsel")
        nc.sync.dma_start(wsel[:], wsel_d[:, :])
        cosw = const.tile([P, WND], F16, name="cosw")
        nc.sync.dma_start(cosw[:], cosw_d[:, :])
        sinw = const.tile([P, WND], F16, name="sinw")
        nc.sync.dma_start(sinw[:], sinw_d[:, :])
        maskw = []
        for n in range(NT):
            mt = const.tile([P, 2 * R], F16, name=f"maskw{n}")
            nc.sync.dma_start(mt[:], maskw_d[n * P:(n + 1) * P, :])
            maskw.append(mt)
        vr0 = const.tile([1, C], F16, name="vr0")
        nc.sync.dma_start(vr0[:], vrows_d[0:1, :])
        vr1 = const.tile([1, C], F16, name="vr1")
        nc.sync.dma_start(vr1[:], vrows_d[1:2, :])
        bvb_sa = const.tile([P, C], F16, name="bvb_sa")
        nc.gpsimd.partition_broadcast(bvb_sa[:], vr0[:])
        bvb_ca = const.tile([P, C], F16, name="bvb_ca")
        nc.gpsimd.partition_broadcast(bvb_ca[:], vr1[:])
        ident = const.tile([P, P], f32, name="ident")
        make_identity(nc, ident)
        identm = const.tile([P, P], F16, name="identm")
        nc.scalar.copy(identm[:], ident[:])
        ones = const.tile([P, 1], f32, name="ones")
        nc.vector.memset(ones[:], 1.0)
        epsT = const.tile([1, 1], f32, name="epsT")
        nc.vector.memset(epsT[:], LN_EPS)

        def pcol(pname, j):
            return params[:, off[pname] + j:off[pname] + j + 1]

        dbg = {}

        def dump(name, ap):
            if not cfg.debug_dump:
                return
            dd = nc.dram_tensor(f"dbg_{name}", list(ap.shape), f32,
                                kind="ExternalOutput")
            if ap.dtype != f32:
                cpy = tmp.tile(list(ap.shape), f32, name="dbgc", bufs=2)
                nc.vector.tensor_copy(cpy[:], ap)
                ap = cpy[:]
            nc.sync.dma_start(dd[:, :], ap)

        # ---------------- input loads (XBAR transpose DMA) ----------------
        mTw = act.tile([P, XW], F16, name="mTw")
        for k in range(KC):
            nc.sync.dma_start_transpose(mTw[:, k * R:(k + 1) * R],
                                        mh_d[:, k * P:(k + 1) * P])
        xTh = tmp.tile([P, XW], F16, name="xTh", bufs=1)
        for k in range(KC):
            nc.sync.dma_start_transpose(xTh[:, k * R:(k + 1) * R],
                                        xh_d[:, k * P:(k + 1) * P])
        xw = act.tile([P, XW], f32, name="xw")
        nc.vector.tensor_copy(xw[:], xTh[:])

        # ---------------- layernorm (transposed layout) ----------------
        def layernorm(gname, bname, hname):
            ps_mu = sps.tile([P, 512], f32, name="gps")
            ps_sq = sps.tile([P, 512], f32, name="gps")
            for k in range(KC):
                ks = slice(k * R, (k + 1) * R)
                sq = tmp.tile([P, R], f32, name="lnsq", bufs=2)
                nc.scalar.square(sq[:], xw[:, ks])
                nc.tensor.matmul(ps_mu[0:1, :R], lhsT=ones[:], rhs=xw[:, ks],
                                 start=(k == 0), stop=(k == KC - 1))
                nc.tensor.matmul(ps_sq[0:1, :R], lhsT=ones[:], rhs=sq[:],
                                 start=(k == 0), stop=(k == KC - 1))
            mu = tmp.tile([1, R], f32, name="ln_mu", bufs=1)
            nc.scalar.mul(mu[:], ps_mu[0:1, :R], 1.0 / C)
            ex2 = tmp.tile([1, R], f32, name="ln_ex2", bufs=1)
            nc.scalar.mul(ex2[:], ps_sq[0:1, :R], 1.0 / C)
            mu2 = tmp.tile([1, R], f32, name="ln_mu2", bufs=1)
            nc.scalar.square(mu2[:], mu[:])
            var = tmp.tile([1, R], f32, name="ln_var", bufs=1)
            nc.vector.tensor_sub(var[:], ex2[:], mu2[:])
            std = tmp.tile([1, R], f32, name="ln_std", bufs=1)
            nc.scalar.activation(std[:], var[:], AF.Sqrt, bias=epsT[:])
            rstd = tmp.tile([1, R], f32, name="ln_rstd", bufs=1)
            nc.vector.reciprocal(rstd[:], std[:])
            mub = tmp.tile([P, R], f32, name="ln_mub", bufs=1)
            nc.gpsimd.partition_broadcast(mub[:], mu[:])
            rstdb = tmp.tile([P, R], f32, name="ln_rstdb", bufs=1)
            nc.gpsimd.partition_broadcast(rstdb[:], rstd[:])
            hw = hpool.tile([P, XW], F16, name=hname, bufs=2)
            for k in range(KC):
                ks = slice(k * R, (k + 1) * R)
                t1 = tmp.tile([P, R], f32, name="ln_cen", bufs=2)
                if k % 2 == 0:
                    nc.gpsimd.tensor_sub(t1[:], xw[:, ks], mub[:])
                    nc.vector.tensor_mul(t1[:], t1[:], rstdb[:])
                else:
                    nc.vector.tensor_sub(t1[:], xw[:, ks], mub[:])
                    nc.gpsimd.tensor_mul(t1[:], t1[:], rstdb[:])
                nc.vector.tensor_scalar(hw[:, ks], t1[:], pcol(gname, k),
                                        pcol(bname, k), op0=OP.mult, op1=OP.add)
            return hw

        # ---------------- GEMM helpers ----------------
        def gemm_ws(w_dram, c0, c1, rhs_slices, evict):
            """out[m, tok] (transposed) = W[:, c0:c1].T @ acts; weights stationary."""
            KT = len(rhs_slices)
            MT = (c1 - c0) // P
            for gi, g0 in enumerate(range(0, MT, GW)):
                gl = min(GW, MT - g0)
                pool = gps if gi % 2 == 0 else sps
                pname = "gps"
                pss = [pool.tile([P, 512], f32, name=pname) for _ in range(gl)]
                for k in range(KT):
                    wt = wpool.tile([P, GW * P], F16, name="wt", bufs=6)
                    nc.sync.dma_start(
                        wt[:, :gl * P],
                        w_dram[k * P:(k + 1) * P, c0 + g0 * P:c0 + (g0 + gl) * P])
                    for j in range(gl):
                        nc.tensor.matmul(pss[j][:, :R],
                                         lhsT=wt[:, j * P:(j + 1) * P],
                                         rhs=rhs_slices[k],
                                         start=(k == 0), stop=(k == KT - 1))
                for j in range(gl):
                    evict(g0 + j, pss[j][:, :R])

        def evict_wide(dst, bname, boff):
            def ev(m, ps):
                ds = dst[:, m * R:(m + 1) * R]
                if m % 2 == 0:
                    nc.scalar.add(ds, ps, pcol(bname, boff + m))
                else:
                    nc.vector.tensor_scalar(ds, ps, pcol(bname, boff + m),
                                            None, op0=OP.add)
            return ev

        def evict_res(bname):
            def ev(m, ps):
                ds = xw[:, m * R:(m + 1) * R]
                nc.vector.scalar_tensor_tensor(ds, ps, pcol(bname, m), ds,
                                               op0=OP.add, op1=OP.add)
            return ev

        def gemm_v(w_dram, c0, hw, vname, bvb):
            """V projection, activation-stationary: out natural [tok, C] fp16."""
            vt = [act.tile([P, C], F16, name=f"{vname}{tt}") for tt in range(NT)]
            pss = {}
            for k in range(KC):
                wt = wpool.tile([P, C], F16, name="wtv", bufs=3)
                nc.sync.dma_start(wt[:], w_dram[k * P:(k + 1) * P, c0:c0 + C])
                for tt in range(NT):
                    for ch in range(2):
                        if k == 0:
                            pool = gps if tt == 0 else sps
                            pss[(tt, ch)] = pool.tile(
                                [P, 512], f32, name="gps")
                        nc.tensor.matmul(
                            pss[(tt, ch)][:, :512],
                            lhsT=hw[:, k * R + tt * P:k * R + (tt + 1) * P],
                            rhs=wt[:, ch * 512:(ch + 1) * 512],
                            start=(k == 0), stop=(k == KC - 1))
            for tt in range(NT):
                for ch in range(2):
                    cs = slice(ch * 512, (ch + 1) * 512)
                    nc.vector.tensor_add(vt[tt][:, cs], pss[(tt, ch)][:, :512],
                                         bvb[:, cs])
            return vt

        # ---------------- feature chains (wide) ----------------
        kfsum = act.tile([P, 2 * NPAIR], f32, name="kfsum")

        def elu1w(srcw, oname, pool, kfbase=None):
            e = tmp.tile([P, WND], F16, name="e_e", bufs=1)
            nc.scalar.activation(e[:], srcw[:], AF.Exp)
            mx = tmp.tile([P, WND], F16, name="e_mx", bufs=1)
            nc.vector.tensor_scalar(mx[:], srcw[:], 0.0, None, op0=OP.max)
            o = pool.tile([P, WND], F16, name=oname)
            if kfbase is None:
                nc.vector.scalar_tensor_tensor(o[:], e[:], 1.0, mx[:],
                                               op0=OP.min, op1=OP.add)
            else:
                for p in range(NPAIR):
                    ps_ = slice(p * R, (p + 1) * R)
                    nc.vector.scalar_tensor_tensor(
                        o[:, ps_], e[:, ps_], 1.0, mx[:, ps_],
                        op0=OP.min, op1=OP.add,
                        accum_out=kfsum[:, kfbase + p:kfbase + p + 1])
            return o

        SHUF = list(range(16, 32)) + list(range(0, 16))

        def ropew(srcw, oname, pool, use_gpsimd=True):
            rot = tmp.tile([P, WND], F16, name="r_rot", bufs=1)
            nc.vector.stream_shuffle(rot[:], srcw[:], mask=SHUF)
            t = tmp.tile([P, WND], F16, name="r_t", bufs=1)
            nc.vector.tensor_mul(t[:], srcw[:], cosw[:])
            eng2 = nc.gpsimd if use_gpsimd else nc.vector
            eng2.tensor_mul(rot[:], rot[:], sinw[:])
            o = pool.tile([P, WND], F16, name=oname)
            nc.vector.tensor_add(o[:], t[:], rot[:])
            return o

        # ---------------- agbuf / kv states ----------------
        o_sst, o_skf = 0, HD * NPAIR
        o_cst, o_ckf = o_skf + NPAIR, o_skf + NPAIR + HD * NPAIR
        agbuf = act.tile([P, AGW], F16, name="agbuf")

        def kv_states(krw, vt, agcol):
            for p in range(NPAIR):
                kn = []
                for tt in range(NT):
                    pt = tps.tile([P, 512], F16, name="tps")
                    nc.tensor.transpose(
                        pt[:P, :P], krw[:, p * R + tt * P:p * R + (tt + 1) * P],
                        identm[:, :])
                    knt = tmp.tile([P, P], F16, name="kn", bufs=4)
                    nc.scalar.copy(knt[:], pt[:P, :P])
                    kn.append(knt)
                st = sps.tile([P, 512], f32, name="gps")
                for h0 in (0, HD):
                    head = 2 * p + h0 // HD
                    for tt in range(NT):
                        nc.tensor.matmul(
                            st[h0:h0 + HD, :HD],
                            lhsT=kn[tt][:, h0:h0 + HD],
                            rhs=vt[tt][:, head * HD:(head + 1) * HD],
                            start=(tt == 0), stop=(tt == NT - 1))
                nc.scalar.copy(agbuf[:, agcol + p * HD:agcol + (p + 1) * HD],
                               st[:, :HD])

        # ================= phase 1: projections + states =================
        h1 = layernorm("ln1_g", "ln1_b", "h1")
        h1s = [h1[:, k * R:(k + 1) * R] for k in range(KC)]
        mTs = [mTw[:, k * R:(k + 1) * R] for k in range(KC)]

        Ktw = tmp.tile([P, WND], F16, name="preact", bufs=2)
        gemm_ws(Wqkv, C, 2 * C, h1s, evict_wide(Ktw, "qkv_b", KC))
        Kf = elu1w(Ktw, "Kf", act, kfbase=0)
        Kr = ropew(Kf, "Kr", act)
        Vt = gemm_v(Wqkv, 2 * C, h1, "Vt", bvb_sa)
        kv_states(Kr, Vt, o_sst)

        K2t = tmp.tile([P, WND], F16, name="preact", bufs=2)
        gemm_ws(Wcakv, 0, C, mTs, evict_wide(K2t, "cakv_b", 0))
        K2f = elu1w(K2t, "K2f", act, kfbase=NPAIR)
        K2r = ropew(K2f, "K2r", act)
        V2t = gemm_v(Wcakv, C, mTw, "V2t", bvb_ca)
        kv_states(K2r, V2t, o_cst)

        nc.scalar.copy(agbuf[:, o_skf:o_skf + NPAIR], kfsum[:, 0:NPAIR])
        nc.scalar.copy(agbuf[:, o_ckf:o_ckf + NPAIR],
                       kfsum[:, NPAIR:2 * NPAIR])

        # ================= AllGather (fp16, groups of 4) =================
        ag_in = dram.tile([P, AGW], F16, name="ag_in")
        ag_out = dram.tile([NC * P, AGW], F16, name="ag_out",
                           addr_space="Shared")
        nc.sync.dma_start(ag_in[:], agbuf[:])
        nc.gpsimd.collective_compute(
            "AllGather", OP.bypass, replica_groups=[list(range(NC))],
            ins=[ag_in[:].opt()], outs=[ag_out[:].opt()])

        # ---------------- Q features + intra-chunk causal attention -------
        Qtw = tmp.tile([P, WND], F16, name="preact", bufs=2)
        gemm_ws(Wqkv, 0, C, h1s, evict_wide(Qtw, "qkv_b", 0))
        Qf = elu1w(Qtw, "Qf", act)
        Qr = ropew(Qf, "Qr", act, use_gpsimd=False)

        yiw = act.tile([P, WND], F16, name="yiw")
        for p in range(NPAIR):
            ps_ = slice(p * R, (p + 1) * R)
            ams = []
            for n in range(NT):
                pa = sps.tile([P, 512], f32, name="gps")
                for h0 in (0, HD):
                    nc.tensor.matmul(
                        pa[:, (h0 // HD) * R:(h0 // HD) * R + R],
                        lhsT=Kr[h0:h0 + HD, p * R + n * P:p * R + (n + 1) * P],
                        rhs=Qr[h0:h0 + HD, ps_],
                        start=True, stop=True, skip_group_check=True)
                am = tmp.tile([P, 2 * R], F16, name="am", bufs=3)
                nc.vector.tensor_mul(am[:], pa[:], maskw[n][:])
                ams.append(am)
            yp = gps.tile([P, 512], f32, name="gps")
            for h0 in (0, HD):
                head = 2 * p + h0 // HD
                for n in range(NT):
                    nc.tensor.matmul(
                        yp[h0:h0 + HD, :R],
                        lhsT=Vt[n][:, head * HD:(head + 1) * HD],
                        rhs=ams[n][:, (h0 // HD) * R:(h0 // HD) * R + R],
                        start=(n == 0), stop=(n == NT - 1))
            nc.scalar.copy(yiw[:, ps_], yp[:, :R])

        # ---------------- gather states, weighted prefix/total ----------
        accP = act.tile([P, AGW], F16, name="accP")
        accT = act.tile([P, AGW], F16, name="accT")
        nc.vector.memset(accP[:], 0.0)
        nc.vector.memset(accT[:], 0.0)
        for r in range(NC):
            agr = tmp.tile([P, AGW], F16, name="agr", bufs=2)
            nc.sync.dma_start(agr[:], ag_out[r * P:(r + 1) * P, :])
            nc.vector.scalar_tensor_tensor(accP[:], agr[:], wsel[:, r:r + 1],
                                           accP[:], op0=OP.mult, op1=OP.add)
            nc.vector.scalar_tensor_tensor(accT[:], agr[:],
                                           wsel[:, NC + r:NC + r + 1],
                                           accT[:], op0=OP.mult, op1=OP.add)

        # ---------------- attention epilogues ----------------
        def attn_out(qf, qr, stcol, kfcol, stacc, yi, oname):
            ow = act.tile([P, WND], F16, name=oname)
            for p in range(NPAIR):
                ps_ = slice(p * R, (p + 1) * R)
                dps = sps.tile([P, 512], f32, name="gps")
                nc.tensor.matmul(dps[0:1, 0:R],
                                 lhsT=accT[0:HD, kfcol + p:kfcol + p + 1],
                                 rhs=qf[0:HD, ps_], start=True, stop=True,
                                 skip_group_check=True)
                nc.tensor.matmul(dps[0:1, R:2 * R],
                                 lhsT=accT[HD:P, kfcol + p:kfcol + p + 1],
                                 rhs=qf[HD:P, ps_], start=True, stop=True,
                                 skip_group_check=True)
                rec = tmp.tile([1, 2 * R], f32, name="rec", bufs=2)
                nc.vector.reciprocal(rec[:], dps[0:1, :2 * R])
                recb = tmp.tile([P, 2 * R], f32, name="recb", bufs=2)
                nc.gpsimd.partition_broadcast(recb[:], rec[:])
                yps = gps.tile([P, 512], f32, name="gps")
                for h0 in (0, HD):
                    nc.tensor.matmul(
                        yps[h0:h0 + HD, :R],
                        lhsT=stacc[h0:h0 + HD,
                                   stcol + p * HD:stcol + (p + 1) * HD],
                        rhs=qr[h0:h0 + HD, ps_],
                        start=True, stop=True, skip_group_check=True)
                if yi is not None:
                    ysum = tmp.tile([P, R], f32, name="ysum", bufs=2)
                    nc.vector.tensor_add(ysum[0:HD, :], yps[0:HD, :R],
                                         yi[0:HD, ps_])
                    nc.vector.tensor_add(ysum[HD:P, :], yps[HD:P, :R],
                                         yi[HD:P, ps_])
                    nc.gpsimd.tensor_mul(ow[0:HD, ps_], ysum[0:HD, :],
                                         recb[0:HD, 0:R])
                    nc.vector.tensor_mul(ow[HD:P, ps_], ysum[HD:P, :],
                                         recb[HD:P, R:2 * R])
                else:
                    nc.vector.tensor_mul(ow[0:HD, ps_], yps[0:HD, :R],
                                         recb[0:HD, 0:R])
                    nc.vector.tensor_mul(ow[HD:P, ps_], yps[HD:P, :R],
                                         recb[HD:P, R:2 * R])
            return ow

        ySA = attn_out(Qf, Qr, o_sst, o_skf, accP, yiw, "ySA")
        ySAs = [ySA[:, k * R:(k + 1) * R] for k in range(KC)]
        gemm_ws(Wsap, 0, C, ySAs, evict_res("sap_b"))
        dump("x1w", xw[:])

        # ================= cross attention =================
        h2 = layernorm("ln2_g", "ln2_b", "h2")
        h2s = [h2[:, k * R:(k + 1) * R] for k in range(KC)]
        Q2t = tmp.tile([P, WND], F16, name="preact", bufs=2)
        gemm_ws(Wcaq, 0, C, h2s, evict_wide(Q2t, "caq_b", 0))
        Q2f = elu1w(Q2t, "Q2f", act)
        Q2r = ropew(Q2f, "Q2r", act, use_gpsimd=False)
        yCA = attn_out(Q2f, Q2r, o_cst, o_ckf, accT, None, "yCA")
        yCAs = [yCA[:, k * R:(k + 1) * R] for k in range(KC)]
        gemm_ws(Wcap, 0, C, yCAs, evict_res("cap_b"))
        dump("x2w", xw[:])

        # ================= MLP =================
        h3 = layernorm("ln3_g", "ln3_b", "h3")
        h3s = [h3[:, k * R:(k + 1) * R] for k in range(KC)]
        gTw = act.tile([P, 4 * KC * R], F16, name="gTw")

        def evict_gelu(m, ps):
            nc.scalar.activation(gTw[:, m * R:(m + 1) * R], ps,
                                 AF.Gelu_apprx_tanh, bias=pcol("fc_b", m))
        gemm_ws(Wfc, 0, 4 * C, h3s, evict_gelu)
        gTs = [gTw[:, k * R:(k + 1) * R] for k in range(4 * KC)]
        gemm_ws(Wfcp, 0, C, gTs, evict_res("fcp_b"))
        dump("xow", xw[:])

        # ================= store (fp16 via XBAR transpose) =================
        for tt in range(NT):
            onat = tmp.tile([P, C], F16, name="onat", bufs=2)
            for k in range(KC):
                pt = sps.tile([P, 512], f32, name="gps")
                nc.tensor.transpose(
                    pt[:P, :P], xw[:, k * R + tt * P:k * R + (tt + 1) * P],
                    ident[:, :])
                nc.scalar.copy(onat[:, k * P:(k + 1) * P], pt[:P, :P])
            nc.sync.dma_start(out_d[tt * P:(tt + 1) * P, :], onat[:])

    nc.compile()
    return nc


# ---------------------------------------------------------------------------
# Entry point
# ---------------------------------------------------------------------------

_CACHE = {}


def _norm_cfg(cfg: Cfg) -> Cfg:
    if cfg.mm not in ("fp16", "bf16"):
        cfg = Cfg(B=cfg.B, T=cfg.T, C=cfg.C, H=cfg.H, NCORE=cfg.NCORE,
                  mm="fp16", gelu=cfg.gelu, debug_dump=cfg.debug_dump)
    return cfg


def _get_program(cfg: Cfg):
    if cfg not in _CACHE:
        _CACHE[cfg] = build_program(cfg)
    return _CACHE[cfg]


def run(inputs, cfg: Cfg = Cfg(), trace: bool = False):
    from concourse.bass_utils import run_bass_kernel_spmd
    cfg = _norm_cfg(cfg)
    nc = _get_program(cfg)
    in_maps = _host_inputs(cfg, inputs)
    res = run_bass_kernel_spmd(nc, in_maps, core_ids=list(range(cfg.NCORE)),
                               trace=trace)
    outs = [np.asarray(res.results[c]["out"], np.float32)
            for c in range(cfg.NCORE)]
    full = np.concatenate(outs, axis=0).reshape(cfg.B, cfg.T, cfg.C)
    return np.asarray(full, np.float32), res


def kernel(**inputs):
    out, _ = run(inputs)
    return out
